# revision 3
# baseline (speedup 1.0000x reference)
"""AdaPT quantized linear (int8-exact via bf16 matmul) on 8 TRN2 NeuronCores.

Reference computes:
    qx = clip(round(x * 127/amax_x), -127, 127)        [N, K] int8
    qw = clip(round(w * 127/amax_w), -127, 127)        [M, K] int8
    out = (qx @ qw.T) / ((127/amax_x)*(127/amax_w)) + bias

Strategy: data-parallel over the 8192-token dim (1024 tokens/core), full
weight on every core, no collectives.  All int8 values are exactly
representable in bf16 (8-bit significand), the PE multiplies bf16 exactly
(products < 2^14) and accumulates in fp32 (sums << 2^24), so the bf16
matmul reproduces the int8 systolic GEMM bit-exactly.  Rounding uses the
+1.5*2^23 magic-constant trick which matches round-half-to-even.

Per-core device kernel (weight-stationary):
  - quantize x.T shard once into resident SBUF bf16 [128, 32, 1024]
  - per m-tile (128 rows of W): DMA w.T block, quantize, 64 accumulating
    matmuls (32 k-tiles x 2 token chunks of 512), dequant+bias on ScalarE
    straight out of PSUM, DMA out.

Output is produced transposed ([M, tokens/core] per core) so the bias can
ride the ScalarE per-partition bias port; host transposes back.
"""

import sys

import numpy as np

sys.path.insert(0, "/opt/trn_rl_repo")

N, K, M = 8192, 4096, 4096
N_CORES = 8
TPC = N // N_CORES  # tokens per core
P = 128
KT = K // P   # 32 k-tiles
MT = M // P   # 32 m-tiles
TF = 512      # matmul moving free dim (one PSUM bank of fp32)
NTF = TPC // TF
XCH = 4       # resident xq is split into chunks for finer scheduling deps
KPC = KT // XCH
MAGIC = float(1.5 * 2**23)  # 12582912.0; +MAGIC then -MAGIC rounds to int (RNE)
MAXV = 127.0


def build(s_x: float, s_w: float, inv_s: float):
    import concourse.mybir as mybir
    import concourse.tile as tile
    from concourse import bacc

    dt = mybir.dt
    AF = mybir.ActivationFunctionType
    OP = mybir.AluOpType

    nc = bacc.Bacc("TRN2", target_bir_lowering=False, debug=False,
                   num_devices=N_CORES)

    # activation() lowers non-Copy bias to a const AP; register the magic
    # rounding constants the same way Bass registers 0.0/1.0 at init.
    for v in (MAGIC, -MAGIC):
        t = nc.alloc_sbuf_tensor(f"const-float32-{v}", [128, 1], dt.float32)
        nc.gpsimd.memset(t.ap(), v)
        nc.const_aps.aps[(dt.float32, v)] = t.ap()
    nc.all_engine_barrier()

    xt = nc.declare_dram_parameter("xt", [K, TPC], dt.float32, isOutput=False)
    wt = nc.declare_dram_parameter("wt", [MT, K, P], dt.float32, isOutput=False)
    bias = nc.declare_dram_parameter("bias", [M], dt.float32, isOutput=False)
    out = nc.declare_dram_parameter("out", [M, TPC], dt.float32, isOutput=True)

    with tile.TileContext(nc) as tc:
        with (
            tc.tile_pool(name="xq", bufs=1) as xq_pool,
            tc.tile_pool(name="xs", bufs=3) as xs_pool,
            tc.tile_pool(name="ws", bufs=3) as ws_pool,
            tc.tile_pool(name="wq", bufs=3) as wq_pool,
            tc.tile_pool(name="cst", bufs=1) as cst_pool,
            tc.tile_pool(name="outp", bufs=3) as out_pool,
            tc.tile_pool(name="ps", bufs=4, space="PSUM") as psum_pool,
        ):
            bias_sb = cst_pool.tile([P, MT], dt.float32, name="bias_sb")
            nc.sync.dma_start(bias_sb[:], bias[:].rearrange("(o p) -> p o", p=P))

            def quantize(dst, src, scale):
                # dst (bf16) = clip(round(src * scale), -127, 127), exactly
                # matching jnp.round (half-to-even) + jnp.clip.
                nc.scalar.activation(src, src, AF.Identity,
                                     bias=MAGIC, scale=scale)
                nc.vector.tensor_scalar(src, src, MAGIC + MAXV, MAGIC - MAXV,
                                        OP.min, OP.max)
                nc.scalar.activation(dst, src, AF.Identity, bias=-MAGIC)

            xq_tiles = [
                xq_pool.tile([P, KPC, TPC], dt.bfloat16, name=f"xq{c}", tag=f"xq{c}")
                for c in range(XCH)
            ]
            for kt in range(KT):
                xs = xs_pool.tile([P, TPC], dt.float32, name="xs")
                nc.sync.dma_start(xs[:], xt[kt * P:(kt + 1) * P, :])
                quantize(xq_tiles[kt // KPC][:, kt % KPC, :], xs[:], s_x)

            for mt in range(MT):
                ws = ws_pool.tile([P, KT, P], dt.float32, name="ws")
                for q in range(XCH):
                    nc.sync.dma_start(
                        ws[:, q * KPC:(q + 1) * KPC, :],
                        wt[mt, q * KPC * P:(q + 1) * KPC * P, :]
                        .rearrange("(o p) f -> p o f", p=P),
                    )
                wq = wq_pool.tile([P, KT, P], dt.bfloat16, name="wq")
                quantize(wq[:], ws[:], s_w)

                pss = [psum_pool.tile([P, TF], dt.float32, name=f"ps{i}") for i in range(NTF)]
                for kt in range(KT):
                    for tf in range(NTF):
                        nc.tensor.matmul(
                            pss[tf][:],
                            wq[:, kt, :],
                            xq_tiles[kt // KPC][:, kt % KPC,
                                               tf * TF:(tf + 1) * TF],
                            start=(kt == 0),
                            stop=(kt == KT - 1),
                        )

                outt = out_pool.tile([P, TPC], dt.float32, name="outt")
                for tf in range(NTF):
                    nc.scalar.activation(
                        outt[:, tf * TF:(tf + 1) * TF], pss[tf][:],
                        AF.Identity, bias=bias_sb[:, mt:mt + 1], scale=inv_s,
                    )
                nc.sync.dma_start(out[mt * P:(mt + 1) * P, :], outt[:])

    nc.compile()
    return nc


def _prep(x, weight, bias, amax_x, amax_w):
    ax = np.float32(np.asarray(amax_x, dtype=np.float32).reshape(-1)[0])
    aw = np.float32(np.asarray(amax_w, dtype=np.float32).reshape(-1)[0])
    s_x = np.float32(127.0) / ax
    s_w = np.float32(127.0) / aw
    inv_s = np.float32(1.0) / (s_x * s_w)

    x = np.asarray(x, dtype=np.float32)
    weight = np.asarray(weight, dtype=np.float32)
    bias = np.asarray(bias, dtype=np.float32)

    xT = np.ascontiguousarray(x.T)  # [K, N]
    # [MT, K, 128]: per m-tile a contiguous k-major block of W^T
    wt3 = np.ascontiguousarray(weight.reshape(MT, P, K).transpose(0, 2, 1))
    in_maps = [
        {
            "xt": np.ascontiguousarray(xT[:, c * TPC:(c + 1) * TPC]),
            "wt": wt3,
            "bias": bias,
        }
        for c in range(N_CORES)
    ]
    return float(s_x), float(s_w), float(inv_s), in_maps


def run(x, weight, bias, amax_x, amax_w, trace: bool = False):
    from concourse.bass_utils import run_bass_kernel_spmd

    s_x, s_w, inv_s, in_maps = _prep(x, weight, bias, amax_x, amax_w)
    nc = build(s_x, s_w, inv_s)
    res = run_bass_kernel_spmd(nc, in_maps, core_ids=list(range(N_CORES)),
                               trace=trace)
    shards = [res.results[c]["out"] for c in range(N_CORES)]
    full = np.concatenate([s.T for s in shards], axis=0).astype(np.float32)
    return full, res


def kernel(x, weight, bias, amax_x, amax_w):
    full, _ = run(x, weight, bias, amax_x, amax_w, trace=False)
    return full


# revision 7
# speedup vs baseline: 1.1866x; 1.1866x over previous
"""AdaPT quantized linear (int8-exact via bf16 matmul) on 8 TRN2 NeuronCores.

Reference computes:
    qx = clip(round(x * 127/amax_x), -127, 127)        [N, K] int8
    qw = clip(round(w * 127/amax_w), -127, 127)        [M, K] int8
    out = (qx @ qw.T) / ((127/amax_x)*(127/amax_w)) + bias

Strategy: data-parallel over the 8192-token dim (1024 tokens/core), full
weight on every core, no collectives.  All int8 values are exactly
representable in bf16 (8-bit significand), the PE multiplies bf16 exactly
(products < 2^14) and accumulates in fp32 (sums << 2^24), so the bf16
matmul reproduces the int8 systolic GEMM bit-exactly.  Rounding uses the
+1.5*2^23 magic-constant trick which matches round-half-to-even.

Per-core device kernel (weight-stationary):
  - quantize x.T shard once into resident SBUF bf16 [128, 32, 1024]
  - per m-tile (128 rows of W): DMA w.T block, quantize, 64 accumulating
    matmuls (32 k-tiles x 2 token chunks of 512), dequant+bias on ScalarE
    straight out of PSUM, DMA out.

Output is produced transposed ([M, tokens/core] per core) so the bias can
ride the ScalarE per-partition bias port; host transposes back.
"""

import sys

import numpy as np

sys.path.insert(0, "/opt/trn_rl_repo")

N, K, M = 8192, 4096, 4096
N_CORES = 8
TPC = N // N_CORES  # tokens per core
P = 128
KT = K // P   # 32 k-tiles
MT = M // P   # 32 m-tiles
TF = 512      # matmul moving free dim (one PSUM bank of fp32)
NTF = TPC // TF
XCH = 4       # resident xq is split into chunks for finer scheduling deps
KPC = KT // XCH
MAGIC = float(1.5 * 2**23)  # 12582912.0; +MAGIC then -MAGIC rounds to int (RNE)
MAXV = 127.0


def build(s_x: float, s_w: float, inv_s: float):
    import concourse.mybir as mybir
    import concourse.tile as tile
    from concourse import bacc

    dt = mybir.dt
    AF = mybir.ActivationFunctionType
    OP = mybir.AluOpType

    nc = bacc.Bacc("TRN2", target_bir_lowering=False, debug=False,
                   num_devices=N_CORES)

    # activation() lowers non-Copy bias to a const AP; register the magic
    # rounding constants the same way Bass registers 0.0/1.0 at init.
    for v in (MAGIC, -MAGIC):
        t = nc.alloc_sbuf_tensor(f"const-float32-{v}", [128, 1], dt.float32)
        nc.gpsimd.memset(t.ap(), v)
        nc.const_aps.aps[(dt.float32, v)] = t.ap()
    nc.all_engine_barrier()

    xt = nc.declare_dram_parameter("xt", [K, TPC], dt.float32, isOutput=False)
    wt = nc.declare_dram_parameter("wt", [MT, K, P], dt.float32, isOutput=False)
    bias = nc.declare_dram_parameter("bias", [M], dt.float32, isOutput=False)
    out = nc.declare_dram_parameter("out", [M, TPC], dt.float32, isOutput=True)

    with tile.TileContext(nc) as tc:
        with (
            tc.tile_pool(name="xq", bufs=1) as xq_pool,
            tc.tile_pool(name="xs", bufs=4) as xs_pool,
            tc.tile_pool(name="ws", bufs=4) as ws_pool,
            tc.tile_pool(name="wq", bufs=4) as wq_pool,
            tc.tile_pool(name="cst", bufs=1) as cst_pool,
            tc.tile_pool(name="outp", bufs=3) as out_pool,
            tc.tile_pool(name="ps", bufs=3, space="PSUM") as psum_pool,
            tc.tile_pool(name="junk", bufs=1) as junk_pool,
            tc.tile_pool(name="psjunk", bufs=1, space="PSUM") as psjunk_pool,
        ):
            bias_sb = cst_pool.tile([P, MT], dt.float32, name="bias_sb")
            nc.sync.dma_start(bias_sb[:], bias[:].rearrange("(o p) -> p o", p=P))

            # PE warmup: junk matmuls keep HAM hot while the first x/w
            # chunks quantize; they retire before any real matmul is ready.
            junk_sb = junk_pool.tile([P, TF], dt.bfloat16, name="junk_sb")
            junk_ps = psjunk_pool.tile([P, TF], dt.float32, name="junk_ps")
            nc.vector.memset(junk_sb[:], 1.0)
            for _ in range(24):
                nc.tensor.matmul(junk_ps[:], junk_sb[:, :P], junk_sb[:],
                                 start=True, stop=True)

            def quantize(dst, src, scale, flip=False):
                # dst (bf16) = clip(round(src * scale), -127, 127), exactly
                # matching jnp.round (half-to-even) + jnp.clip.  The clip
                # (min/max pair) only runs on DVE; the first affine pass
                # alternates between ScalarE and DVE so a pair of tiles
                # costs each engine 3 passes.
                if flip:
                    nc.vector.tensor_scalar(src, src, scale, MAGIC,
                                            OP.mult, OP.add)
                else:
                    nc.scalar.activation(src, src, AF.Identity,
                                         bias=MAGIC, scale=scale)
                nc.vector.tensor_scalar(src, src, MAGIC + MAXV, MAGIC - MAXV,
                                        OP.min, OP.max)
                nc.scalar.activation(dst, src, AF.Identity, bias=-MAGIC)

            xq_tiles = [
                xq_pool.tile([P, KPC, TPC], dt.bfloat16, name=f"xq{c}", tag=f"xq{c}")
                for c in range(XCH)
            ]

            def quant_x(kt):
                xs = xs_pool.tile([P, TPC], dt.float32, name="xs")
                nc.sync.dma_start(xs[:], xt[kt * P:(kt + 1) * P, :])
                quantize(xq_tiles[kt // KPC][:, kt % KPC, :], xs[:], s_x,
                         flip=(kt % 2 == 1))

            def prep_w(mt):
                ws = ws_pool.tile([P, KT, P], dt.float32, name="ws")
                for q in range(XCH):
                    nc.sync.dma_start(
                        ws[:, q * KPC:(q + 1) * KPC, :],
                        wt[mt, q * KPC * P:(q + 1) * KPC * P, :]
                        .rearrange("(o p) f -> p o f", p=P),
                    )
                wq = wq_pool.tile([P, KT, P], dt.bfloat16, name="wq")
                quantize(wq[:], ws[:], s_w, flip=False)
                return wq

            def alloc_ps():
                return [psum_pool.tile([P, TF], dt.float32, name=f"ps{i}")
                        for i in range(NTF)]

            def mm(pss, wq, kt, start, stop):
                for tf in range(NTF):
                    nc.tensor.matmul(
                        pss[tf][:],
                        wq[:, kt, :],
                        xq_tiles[kt // KPC][:, kt % KPC,
                                           tf * TF:(tf + 1) * TF],
                        start=start, stop=stop,
                    )

            def store(mt, pss):
                outt = out_pool.tile([P, TPC], dt.float32, name="outt")
                for tf in range(NTF):
                    nc.vector.tensor_scalar(
                        outt[:, tf * TF:(tf + 1) * TF], pss[tf][:],
                        inv_s, bias_sb[:, mt:mt + 1], OP.mult, OP.add,
                    )
                nc.sync.dma_start(out[mt * P:(mt + 1) * P, :], outt[:])

            # Fused prologue: quantize x chunk-by-chunk; after each chunk,
            # run that chunk's matmuls for the first PRO m-tiles so the PE
            # has work long before the full x is quantized (psum k-order is
            # free).  3 m-tiles x 2 psum banks + 1 junk bank fit in PSUM.
            PRO = min(3, MT)
            wqs = [prep_w(mt) for mt in range(PRO)]
            pro_ps = [alloc_ps() for _ in range(PRO)]
            for c in range(XCH):
                for k in range(KPC):
                    quant_x(c * KPC + k)
                for mt in range(PRO):
                    for k in range(KPC):
                        kt = c * KPC + k
                        mm(pro_ps[mt], wqs[mt], kt,
                           start=(kt == 0), stop=(kt == KT - 1))
            for mt in range(PRO):
                store(mt, pro_ps[mt])

            # Steady-state m-loop.
            for mt in range(PRO, MT):
                wq = prep_w(mt)
                pss = alloc_ps()
                for kt in range(KT):
                    mm(pss, wq, kt, start=(kt == 0), stop=(kt == KT - 1))
                store(mt, pss)

    nc.compile()
    return nc


def _prep(x, weight, bias, amax_x, amax_w):
    ax = np.float32(np.asarray(amax_x, dtype=np.float32).reshape(-1)[0])
    aw = np.float32(np.asarray(amax_w, dtype=np.float32).reshape(-1)[0])
    s_x = np.float32(127.0) / ax
    s_w = np.float32(127.0) / aw
    inv_s = np.float32(1.0) / (s_x * s_w)

    x = np.asarray(x, dtype=np.float32)
    weight = np.asarray(weight, dtype=np.float32)
    bias = np.asarray(bias, dtype=np.float32)

    xT = np.ascontiguousarray(x.T)  # [K, N]
    # [MT, K, 128]: per m-tile a contiguous k-major block of W^T
    wt3 = np.ascontiguousarray(weight.reshape(MT, P, K).transpose(0, 2, 1))
    in_maps = [
        {
            "xt": np.ascontiguousarray(xT[:, c * TPC:(c + 1) * TPC]),
            "wt": wt3,
            "bias": bias,
        }
        for c in range(N_CORES)
    ]
    return float(s_x), float(s_w), float(inv_s), in_maps


def run(x, weight, bias, amax_x, amax_w, trace: bool = False):
    from concourse.bass_utils import run_bass_kernel_spmd

    s_x, s_w, inv_s, in_maps = _prep(x, weight, bias, amax_x, amax_w)
    nc = build(s_x, s_w, inv_s)
    res = run_bass_kernel_spmd(nc, in_maps, core_ids=list(range(N_CORES)),
                               trace=trace)
    shards = [res.results[c]["out"] for c in range(N_CORES)]
    full = np.concatenate([s.T for s in shards], axis=0).astype(np.float32)
    return full, res


def kernel(x, weight, bias, amax_x, amax_w):
    full, _ = run(x, weight, bias, amax_x, amax_w, trace=False)
    return full
